# revision 19
# baseline (speedup 1.0000x reference)
"""BinaryConnect 3x3 SAME conv (NHWC, 32x112x112x128 -> 32x112x112x256) on 8 trn2 cores.

Strategy (data-parallel, 4 images per core):
  - Host: binarize kernel to +/-1 fp16 (exact), cast x to fp16, transpose to
    channel-major [cin, n, hp, wp] with a 1-px zero halo (115x114 rows incl.
    one zero tail row).
  - Device: for each output tile of 4 rows x 112 cols (one cout half), the
    conv is 9 accumulating matmuls (one per 3x3 tap):
      lhsT = wb[tap] [cin=128, cout_half=128]   (stationary),
      rhs  = x[cin=128, rows r0+dh : r0+dh+4, cols dw : dw+112] (2D-AP moving,
             N = 448), fp16 in, fp32 PSUM accumulate.
    Output is written channel-major [cout, n, h*112+w] and un-transposed on
    the host. PE warmup matmuls on a memset tile un-throttle the HAM clock
    gate while the first input band DMA is still in flight.
"""

import os

import numpy as np

import concourse.bass as bass
import concourse.mybir as mybir
import concourse.tile as tile
from concourse import bacc
from concourse.bass_utils import run_bass_kernel_spmd

N_CORES = 8
NPC = 4            # images per core
H = 112
WP = 114           # padded row width
HP = 115           # 1 top pad + 112 rows + 1 bottom pad + 1 zero tail row
CI = 128
CO = 256
TROWS = 4          # output rows per matmul tile
S = TROWS * H      # 448 matmul free dim (<=512 fp32 PSUM bank)
BROWS = 28         # output rows per input band
NB = H // BROWS    # 4 bands per image
BIN = BROWS + 3    # input rows per band incl. halo
TSB = BROWS // TROWS  # 7 tiles per band

_nc_cache = None
LAST_RESULT = None


def _build():
    nc = bacc.Bacc(
        "TRN2",
        target_bir_lowering=False,
        debug=False,
        num_devices=N_CORES,
    )
    x_d = nc.dram_tensor(
        "xp", [CI, NPC, HP, WP], mybir.dt.float16, kind="ExternalInput"
    )
    w_d = nc.dram_tensor(
        "wt", [CI, 2, 9 * 128], mybir.dt.float16, kind="ExternalInput"
    )
    o_d = nc.dram_tensor(
        "out_cm", [CO, NPC, H * H], mybir.dt.float32, kind="ExternalOutput"
    )
    with tile.TileContext(nc) as tc:
        with (
            tc.tile_pool(name="xpool", bufs=1) as xpool,
            tc.tile_pool(name="wpool", bufs=1) as wpool,
            tc.tile_pool(name="psum", bufs=8, space=bass.MemorySpace.PSUM) as psum,
            tc.tile_pool(name="opool", bufs=12) as opool,
        ):
            # Warmup operand with no DMA dependency: memset, so the PE warmup
            # (HAM un-throttle) can start right after the framework preamble,
            # overlapping the input DMA latency.
            wta = wpool.tile([CI, S], mybir.dt.float16, tag="wta", name="wta")
            nc.gpsimd.memset(wta[:], 0.0)
            # Weights split by cout half (separate tiles, so the first matmul
            # group gates on only a 295KB DMA); first-chunk input in between.
            wt0 = wpool.tile([CI, 9 * 128], mybir.dt.float16, tag="wt0", name="wt0")
            nc.sync.dma_start(wt0[:], w_d[:, 0, :])
            # Small first chunk of image 0 (rows 0-7) so the first real
            # matmul group (st=0) gates on ~230KB instead of a full band.
            # On the ACT ring: completion receipts serialize per HWDGE ring,
            # so keeping xa off the sync ring lets its sem fire independently
            # of the weight DMAs'.
            xa = xpool.tile([CI, 8, WP], mybir.dt.float16, tag="xa", name="xa")
            nc.scalar.dma_start(xa[:], x_d[:, 0, 0:8, :])
            wt1 = wpool.tile([CI, 9 * 128], mybir.dt.float16, tag="wt1", name="wt1")
            nc.sync.dma_start(wt1[:], w_d[:, 1, :])
            wt_h = [wt0, wt1]
            # PE warmup: 9 throwaway matmuls to push the HAM activity window
            # to K=8/8 before the real stream begins.
            wu = psum.tile([128, S], mybir.dt.float32, name="ps")
            for _ in range(9):
                nc.tensor.matmul(
                    wu[:], wta[:, 0:128], wta[:, 0:S], start=True, stop=True
                )
            # Band-split the input (4 bands of 28 output rows per image, 31
            # input rows each incl. halo) so compute gates on ~900KB chunks.
            xs = {}
            for n in range(NPC):
                for b in range(NB):
                    xt = xpool.tile(
                        [CI, BIN, WP],
                        mybir.dt.float16,
                        tag=f"x{n}_{b}",
                        name=f"x{n}_{b}",
                    )
                    nc.sync.dma_start(
                        xt[:], x_d[:, n, b * BROWS : b * BROWS + BIN, :]
                    )
                    xs[n, b] = xt
            for n in range(NPC):
                for b in range(NB):
                    for st in range(TSB):
                        r0 = st * TROWS       # band-relative top output row
                        o0 = (b * BROWS + st * TROWS) * H
                        xsrc = xa if (n, b, st) == (0, 0, 0) else xs[n, b]
                        for half in range(2):
                            ps = psum.tile([128, S], mybir.dt.float32, name="ps")
                            t = 0
                            for dh in range(3):
                                for dw in range(3):
                                    w0 = t * 128
                                    nc.tensor.matmul(
                                        ps[:],
                                        wt_h[half][:, w0 : w0 + 128],
                                        xsrc[
                                            :,
                                            r0 + dh : r0 + dh + TROWS,
                                            dw : dw + H,
                                        ],
                                        start=(t == 0),
                                        stop=(t == 8),
                                    )
                                    t += 1
                            ot = opool.tile([128, S], mybir.dt.float32, name="ot")
                            nc.vector.tensor_copy(ot[:], ps[:])
                            # ACT's HWDGE ring — keeps output DMAs off the
                            # sync ring so they don't queue behind input DMAs.
                            nc.scalar.dma_start(
                                o_d[half * 128 : half * 128 + 128, n, o0 : o0 + S],
                                ot[:],
                            )
    nc.compile()
    return nc


def _get_nc():
    global _nc_cache
    if _nc_cache is None:
        _nc_cache = _build()
    return _nc_cache


def kernel(x, kernel):
    global LAST_RESULT
    x = np.asarray(x)
    k = np.asarray(kernel)

    # wt[ci, half, tap*128 + co'] = sign(kernel[dh, dw, ci, half*128 + co'])
    wb = np.where(k >= 0, np.float16(1), np.float16(-1))  # [3,3,128,256]
    wt = np.ascontiguousarray(
        wb.transpose(2, 0, 1, 3)          # [ci, dh, dw, co]
        .reshape(CI, 9, 2, 128)           # co -> (half, co')
        .transpose(0, 2, 1, 3)            # [ci, half, tap, co']
        .reshape(CI, 2, 9 * 128)
    )

    x16 = x.astype(np.float16)  # [32,112,112,128]
    in_maps = []
    for c in range(N_CORES):
        xp = np.zeros((CI, NPC, HP, WP), np.float16)
        xp[:, :, 1:113, 1:113] = x16[c * NPC : (c + 1) * NPC].transpose(3, 0, 1, 2)
        in_maps.append({"xp": xp, "wt": wt})

    nc = _get_nc()
    trace = os.environ.get("BCONV_TRACE", "0") == "1"
    kwargs = {}
    if trace and os.environ.get("BCONV_TRACE_CORES", "") == "all":
        kwargs["trace_cores"] = list(range(N_CORES))
    res = run_bass_kernel_spmd(
        nc, in_maps, core_ids=list(range(N_CORES)), trace=trace, **kwargs
    )
    LAST_RESULT = res

    out = np.empty((32, H, H, CO), np.float32)
    for c in range(N_CORES):
        o = res.results[c]["out_cm"].reshape(CO, NPC, H, H)
        out[c * NPC : (c + 1) * NPC] = o.transpose(1, 2, 3, 0)
    return out


# revision 20
# speedup vs baseline: 1.0015x; 1.0015x over previous
"""BinaryConnect 3x3 SAME conv (NHWC, 32x112x112x128 -> 32x112x112x256) on 8 trn2 cores.

Strategy (data-parallel, 4 images per core):
  - Host: binarize kernel to +/-1 fp16 (exact), cast x to fp16, transpose to
    channel-major [cin, n, hp, wp] with a 1-px zero halo (115x114 rows incl.
    one zero tail row).
  - Device: for each output tile of 4 rows x 112 cols (one cout half), the
    conv is 9 accumulating matmuls (one per 3x3 tap):
      lhsT = wb[tap] [cin=128, cout_half=128]   (stationary),
      rhs  = x[cin=128, rows r0+dh : r0+dh+4, cols dw : dw+112] (2D-AP moving,
             N = 448), fp16 in, fp32 PSUM accumulate.
    Output is written channel-major [cout, n, h*112+w] and un-transposed on
    the host. PE warmup matmuls on a memset tile un-throttle the HAM clock
    gate while the first input band DMA is still in flight.
"""

import os

import numpy as np

import concourse.bass as bass
import concourse.mybir as mybir
import concourse.tile as tile
from concourse import bacc
from concourse.bass_utils import run_bass_kernel_spmd

N_CORES = 8
NPC = 4            # images per core
H = 112
WP = 114           # padded row width
HP = 115           # 1 top pad + 112 rows + 1 bottom pad + 1 zero tail row
CI = 128
CO = 256
TROWS = 4          # output rows per matmul tile
S = TROWS * H      # 448 matmul free dim (<=512 fp32 PSUM bank)
BROWS = 28         # output rows per input band
NB = H // BROWS    # 4 bands per image
BIN = BROWS + 3    # input rows per band incl. halo
TSB = BROWS // TROWS  # 7 tiles per band

_nc_cache = None
LAST_RESULT = None


def _build():
    nc = bacc.Bacc(
        "TRN2",
        target_bir_lowering=False,
        debug=False,
        num_devices=N_CORES,
    )
    x_d = nc.dram_tensor(
        "xp", [CI, NPC, HP, WP], mybir.dt.float16, kind="ExternalInput"
    )
    w_d = nc.dram_tensor(
        "wt", [CI, 2, 9 * 128], mybir.dt.float16, kind="ExternalInput"
    )
    o_d = nc.dram_tensor(
        "out_cm", [CO, NPC, H * H], mybir.dt.float32, kind="ExternalOutput"
    )
    with tile.TileContext(nc) as tc:
        with (
            tc.tile_pool(name="xpool", bufs=1) as xpool,
            tc.tile_pool(name="wpool", bufs=1) as wpool,
            tc.tile_pool(name="psum", bufs=8, space=bass.MemorySpace.PSUM) as psum,
            tc.tile_pool(name="opool", bufs=12) as opool,
        ):
            # Warmup operand with no DMA dependency: memset, so the PE warmup
            # (HAM un-throttle) can start right after the framework preamble,
            # overlapping the input DMA latency.
            wta = wpool.tile([CI, S], mybir.dt.float16, tag="wta", name="wta")
            nc.gpsimd.memset(wta[:], 0.0)
            # Weights split by cout half (separate tiles, so the first matmul
            # group gates on only a 295KB DMA); first-chunk input in between.
            wt0 = wpool.tile([CI, 9 * 128], mybir.dt.float16, tag="wt0", name="wt0")
            nc.sync.dma_start(wt0[:], w_d[:, 0, :])
            # Small first chunk of image 0 (rows 0-7) so the first real
            # matmul group (st=0) gates on ~230KB instead of a full band.
            # On the ACT ring: completion receipts serialize per HWDGE ring,
            # so keeping xa off the sync ring lets its sem fire independently
            # of the weight DMAs'.
            xa = xpool.tile([CI, 8, WP], mybir.dt.float16, tag="xa", name="xa")
            nc.scalar.dma_start(xa[:], x_d[:, 0, 0:8, :])
            wt1 = wpool.tile([CI, 9 * 128], mybir.dt.float16, tag="wt1", name="wt1")
            nc.sync.dma_start(wt1[:], w_d[:, 1, :])
            wt_h = [wt0, wt1]
            # PE warmup: 9 throwaway matmuls to push the HAM activity window
            # to K=8/8 before the real stream begins.
            wu = psum.tile([128, S], mybir.dt.float32, name="ps")
            for _ in range(9):
                nc.tensor.matmul(
                    wu[:], wta[:, 0:128], wta[:, 0:S], start=True, stop=True
                )
            # Band-split the input (4 bands of 28 output rows per image, 31
            # input rows each incl. halo) so compute gates on ~900KB chunks.
            xs = {}
            for n in range(NPC):
                for b in range(NB):
                    xt = xpool.tile(
                        [CI, BIN, WP],
                        mybir.dt.float16,
                        tag=f"x{n}_{b}",
                        name=f"x{n}_{b}",
                    )
                    nc.sync.dma_start(
                        xt[:], x_d[:, n, b * BROWS : b * BROWS + BIN, :]
                    )
                    xs[n, b] = xt
            # Spatial tiles are processed in pairs per output DMA: one DMA
            # covering 2 tiles doubles the per-partition contiguous run
            # (1.8KB -> 3.6KB packets), halving the SDMA packet count the
            # output queue must drain (it otherwise backlogs ~8us at the end).
            pairs = [(0, 1), (2, 3), (4, 5), (6,)]
            for n in range(NPC):
                for b in range(NB):
                    for sts in pairs:
                        for half in range(2):
                            ot = opool.tile(
                                [128, 2 * S], mybir.dt.float32, name="ot"
                            )
                            for j, st in enumerate(sts):
                                r0 = st * TROWS  # band-relative top output row
                                xsrc = (
                                    xa if (n, b, st) == (0, 0, 0) else xs[n, b]
                                )
                                ps = psum.tile(
                                    [128, S], mybir.dt.float32, name="ps"
                                )
                                t = 0
                                for dh in range(3):
                                    for dw in range(3):
                                        w0 = t * 128
                                        nc.tensor.matmul(
                                            ps[:],
                                            wt_h[half][:, w0 : w0 + 128],
                                            xsrc[
                                                :,
                                                r0 + dh : r0 + dh + TROWS,
                                                dw : dw + H,
                                            ],
                                            start=(t == 0),
                                            stop=(t == 8),
                                        )
                                        t += 1
                                nc.vector.tensor_copy(
                                    ot[:, j * S : (j + 1) * S], ps[:]
                                )
                            width = len(sts) * S
                            o0 = (b * BROWS + sts[0] * TROWS) * H
                            # ACT's HWDGE ring — keeps output DMAs off the
                            # sync ring so they don't queue behind input DMAs.
                            nc.scalar.dma_start(
                                o_d[
                                    half * 128 : half * 128 + 128,
                                    n,
                                    o0 : o0 + width,
                                ],
                                ot[:, 0:width],
                            )
    nc.compile()
    return nc


def _get_nc():
    global _nc_cache
    if _nc_cache is None:
        _nc_cache = _build()
    return _nc_cache


def kernel(x, kernel):
    global LAST_RESULT
    x = np.asarray(x)
    k = np.asarray(kernel)

    # wt[ci, half, tap*128 + co'] = sign(kernel[dh, dw, ci, half*128 + co'])
    wb = np.where(k >= 0, np.float16(1), np.float16(-1))  # [3,3,128,256]
    wt = np.ascontiguousarray(
        wb.transpose(2, 0, 1, 3)          # [ci, dh, dw, co]
        .reshape(CI, 9, 2, 128)           # co -> (half, co')
        .transpose(0, 2, 1, 3)            # [ci, half, tap, co']
        .reshape(CI, 2, 9 * 128)
    )

    x16 = x.astype(np.float16)  # [32,112,112,128]
    in_maps = []
    for c in range(N_CORES):
        xp = np.zeros((CI, NPC, HP, WP), np.float16)
        xp[:, :, 1:113, 1:113] = x16[c * NPC : (c + 1) * NPC].transpose(3, 0, 1, 2)
        in_maps.append({"xp": xp, "wt": wt})

    nc = _get_nc()
    trace = os.environ.get("BCONV_TRACE", "0") == "1"
    kwargs = {}
    if trace and os.environ.get("BCONV_TRACE_CORES", "") == "all":
        kwargs["trace_cores"] = list(range(N_CORES))
    res = run_bass_kernel_spmd(
        nc, in_maps, core_ids=list(range(N_CORES)), trace=trace, **kwargs
    )
    LAST_RESULT = res

    out = np.empty((32, H, H, CO), np.float32)
    for c in range(N_CORES):
        o = res.results[c]["out_cm"].reshape(CO, NPC, H, H)
        out[c * NPC : (c + 1) * NPC] = o.transpose(1, 2, 3, 0)
    return out


# revision 21
# speedup vs baseline: 1.0044x; 1.0028x over previous
"""BinaryConnect 3x3 SAME conv (NHWC, 32x112x112x128 -> 32x112x112x256) on 8 trn2 cores.

Strategy (data-parallel, 4 images per core):
  - Host: binarize kernel to +/-1 fp16 (exact), cast x to fp16, transpose to
    channel-major [cin, n, hp, wp] with a 1-px zero halo (115x114 rows incl.
    one zero tail row).
  - Device: for each output tile of 4 rows x 112 cols (one cout half), the
    conv is 9 accumulating matmuls (one per 3x3 tap):
      lhsT = wb[tap] [cin=128, cout_half=128]   (stationary),
      rhs  = x[cin=128, rows r0+dh : r0+dh+4, cols dw : dw+112] (2D-AP moving,
             N = 448), fp16 in, fp32 PSUM accumulate.
    Output is written channel-major [cout, n, h*112+w] and un-transposed on
    the host. PE warmup matmuls on a memset tile un-throttle the HAM clock
    gate while the first input band DMA is still in flight.
"""

import os

import numpy as np

import concourse.bass as bass
import concourse.mybir as mybir
import concourse.tile as tile
from concourse import bacc
from concourse.bass_utils import run_bass_kernel_spmd

N_CORES = 8
NPC = 4            # images per core
H = 112
WP = 114           # padded row width
HP = 115           # 1 top pad + 112 rows + 1 bottom pad + 1 zero tail row
CI = 128
CO = 256
TROWS = 4          # output rows per matmul tile
S = TROWS * H      # 448 matmul free dim (<=512 fp32 PSUM bank)
BROWS = 28         # output rows per input band
NB = H // BROWS    # 4 bands per image
BIN = BROWS + 3    # input rows per band incl. halo
TSB = BROWS // TROWS  # 7 tiles per band

_nc_cache = None
LAST_RESULT = None


def _build():
    nc = bacc.Bacc(
        "TRN2",
        target_bir_lowering=False,
        debug=False,
        num_devices=N_CORES,
    )
    x_d = nc.dram_tensor(
        "xp", [CI, NPC, HP, WP], mybir.dt.float16, kind="ExternalInput"
    )
    w_d = nc.dram_tensor(
        "wt", [CI, 2, 9 * 128], mybir.dt.float16, kind="ExternalInput"
    )
    o_d = nc.dram_tensor(
        "out_cm", [CO, NPC, H * H], mybir.dt.float32, kind="ExternalOutput"
    )
    with tile.TileContext(nc) as tc:
        with (
            tc.tile_pool(name="xpool", bufs=1) as xpool,
            tc.tile_pool(name="wpool", bufs=1) as wpool,
            tc.tile_pool(name="psum", bufs=8, space=bass.MemorySpace.PSUM) as psum,
            tc.tile_pool(name="opool", bufs=12) as opool,
        ):
            # Warmup operand with no DMA dependency: memset, so the PE warmup
            # (HAM un-throttle) can start right after the framework preamble,
            # overlapping the input DMA latency.
            wta = wpool.tile([CI, S], mybir.dt.float16, tag="wta", name="wta")
            nc.gpsimd.memset(wta[:], 0.0)
            # Weights split by cout half (separate tiles, so the first matmul
            # group gates on only a 295KB DMA); first-chunk input in between.
            wt0 = wpool.tile([CI, 9 * 128], mybir.dt.float16, tag="wt0", name="wt0")
            nc.sync.dma_start(wt0[:], w_d[:, 0, :])
            # Small first chunk of image 0 (rows 0-7) so the first real
            # matmul group (st=0) gates on ~230KB instead of a full band.
            # On the ACT ring: completion receipts serialize per HWDGE ring,
            # so keeping xa off the sync ring lets its sem fire independently
            # of the weight DMAs'.
            xa = xpool.tile([CI, 8, WP], mybir.dt.float16, tag="xa", name="xa")
            nc.scalar.dma_start(xa[:], x_d[:, 0, 0:8, :])
            wt1 = wpool.tile([CI, 9 * 128], mybir.dt.float16, tag="wt1", name="wt1")
            nc.sync.dma_start(wt1[:], w_d[:, 1, :])
            wt_h = [wt0, wt1]
            # PE warmup: 9 throwaway matmuls to push the HAM activity window
            # to K=8/8 before the real stream begins.
            wu = psum.tile([128, S], mybir.dt.float32, name="ps")
            for _ in range(9):
                nc.tensor.matmul(
                    wu[:], wta[:, 0:128], wta[:, 0:S], start=True, stop=True
                )
            # Band-split the input (4 bands of 28 output rows per image, 31
            # input rows each incl. halo) so compute gates on ~900KB chunks.
            xs = {}
            for n in range(NPC):
                for b in range(NB):
                    xt = xpool.tile(
                        [CI, BIN, WP],
                        mybir.dt.float16,
                        tag=f"x{n}_{b}",
                        name=f"x{n}_{b}",
                    )
                    nc.sync.dma_start(
                        xt[:], x_d[:, n, b * BROWS : b * BROWS + BIN, :]
                    )
                    xs[n, b] = xt
            # Spatial tiles are processed in pairs per output DMA: one DMA
            # covering 2 tiles doubles the per-partition contiguous run
            # (1.8KB -> 3.6KB packets), halving the SDMA packet count the
            # output queue must drain (it otherwise backlogs ~8us at the end).
            pairs = [(0, 1), (2, 3), (4, 5), (6,)]

            def emit_group(n, b, st, half, ot, j):
                r0 = st * TROWS  # band-relative top output row
                xsrc = xa if (n, b, st) == (0, 0, 0) else xs[n, b]
                ps = psum.tile([128, S], mybir.dt.float32, name="ps")
                t = 0
                for dh in range(3):
                    for dw in range(3):
                        nc.tensor.matmul(
                            ps[:],
                            wt_h[half][:, t * 128 : t * 128 + 128],
                            xsrc[:, r0 + dh : r0 + dh + TROWS, dw : dw + H],
                            start=(t == 0),
                            stop=(t == 8),
                        )
                        t += 1
                nc.vector.tensor_copy(ot[:, j * S : (j + 1) * S], ps[:])

            def emit_dma(n, b, half, sts, ot):
                width = len(sts) * S
                o0 = (b * BROWS + sts[0] * TROWS) * H
                # ACT's HWDGE ring — keeps output DMAs off the sync ring so
                # they don't queue behind input DMAs.
                nc.scalar.dma_start(
                    o_d[half * 128 : half * 128 + 128, n, o0 : o0 + width],
                    ot[:, 0:width],
                )

            for n in range(NPC):
                for b in range(NB):
                    for sts in pairs:
                        if (n, b, sts) == (0, 0, (0, 1)):
                            # First pair: run both halves of st=0 (gated only
                            # on the small prefetched xa chunk) before st=1
                            # (gated on the full first band DMA).
                            ots = [
                                opool.tile([128, 2 * S], mybir.dt.float32, name="ot")
                                for _ in range(2)
                            ]
                            for j, half in [(0, 0), (0, 1), (1, 0), (1, 1)]:
                                emit_group(n, b, sts[j], half, ots[half], j)
                            for half in range(2):
                                emit_dma(n, b, half, sts, ots[half])
                        else:
                            for half in range(2):
                                ot = opool.tile(
                                    [128, 2 * S], mybir.dt.float32, name="ot"
                                )
                                for j, st in enumerate(sts):
                                    emit_group(n, b, st, half, ot, j)
                                emit_dma(n, b, half, sts, ot)
    nc.compile()
    return nc


def _get_nc():
    global _nc_cache
    if _nc_cache is None:
        _nc_cache = _build()
    return _nc_cache


def kernel(x, kernel):
    global LAST_RESULT
    x = np.asarray(x)
    k = np.asarray(kernel)

    # wt[ci, half, tap*128 + co'] = sign(kernel[dh, dw, ci, half*128 + co'])
    wb = np.where(k >= 0, np.float16(1), np.float16(-1))  # [3,3,128,256]
    wt = np.ascontiguousarray(
        wb.transpose(2, 0, 1, 3)          # [ci, dh, dw, co]
        .reshape(CI, 9, 2, 128)           # co -> (half, co')
        .transpose(0, 2, 1, 3)            # [ci, half, tap, co']
        .reshape(CI, 2, 9 * 128)
    )

    x16 = x.astype(np.float16)  # [32,112,112,128]
    in_maps = []
    for c in range(N_CORES):
        xp = np.zeros((CI, NPC, HP, WP), np.float16)
        xp[:, :, 1:113, 1:113] = x16[c * NPC : (c + 1) * NPC].transpose(3, 0, 1, 2)
        in_maps.append({"xp": xp, "wt": wt})

    nc = _get_nc()
    trace = os.environ.get("BCONV_TRACE", "0") == "1"
    kwargs = {}
    if trace and os.environ.get("BCONV_TRACE_CORES", "") == "all":
        kwargs["trace_cores"] = list(range(N_CORES))
    res = run_bass_kernel_spmd(
        nc, in_maps, core_ids=list(range(N_CORES)), trace=trace, **kwargs
    )
    LAST_RESULT = res

    out = np.empty((32, H, H, CO), np.float32)
    for c in range(N_CORES):
        o = res.results[c]["out_cm"].reshape(CO, NPC, H, H)
        out[c * NPC : (c + 1) * NPC] = o.transpose(1, 2, 3, 0)
    return out
